# revision 8
# baseline (speedup 1.0000x reference)
"""Multi-headed self-attention (B=2, S=2048, D=1024, H=16) on 8 TRN2 cores.

Sharding: hybrid batch x head tensor-parallel. Core c handles batch c//4 and
heads (c%4)*4 .. (c%4)*4+3 (two head-pairs). Host sums the 4 partials per
batch.

Key design points (vs the f32r feature-major baseline):
- x = query + pos_emb is computed on host (fp32) and shipped transposed, so
  pos_emb is never transferred and no device add is needed.
- QKV projection and QK^T run in f32r (full-rate fp32) for accuracy; the PE
  streams f32r at the same 1 row/cycle as bf16 for moving sizes >= 256.
- Attention weights P = exp(scores) and V are bf16. AV runs token-major: P is
  the stationary operand and V a 65-column moving operand whose last column is
  ones, so softmax denominators land in psum column 64 and normalization is a
  per-partition reciprocal + scale on the vector engine (no cross-partition
  broadcast round-trip through DRAM).
- Attention is processed in 8 jobs = (head, query-half). Each job's P tiles
  [128 keys x 16 kb x 1024 q] are double-buffered so job N's AV (P stationary,
  29ns/matmul incl. overlapped ldweights) runs while job N+1's QK/exp stream;
  the scalar engine's exp (~1 elem/lane/cycle) is the pacing engine.
- AV output is token-major [q, dims]; a bf16 PE transpose per 128x128 block
  restores feature-major oT for the output projection, borrowing QK psum
  slots to stay within the 8 psum banks.
"""

import os
import sys

import numpy as np

if "/opt/trn_rl_repo" not in sys.path:
    sys.path.insert(0, "/opt/trn_rl_repo")

B, S, D, H = 2, 2048, 1024, 16
DK = 64
P = 128
NCORES = 8
HPC = H // (NCORES // B)  # heads per core = 4
T = S  # tokens per core (one batch)
NDC = D // P  # 8 contraction chunks
NTB = T // P  # 16 token blocks
SCALE = DK**-0.5

_CACHE = {}


def _build_program(reps=1):
    from contextlib import ExitStack, nullcontext

    import concourse.bass as bass
    import concourse.tile as tile
    from concourse import bacc
    from concourse import mybir
    from concourse.masks import make_identity

    f32 = mybir.dt.float32
    f32r = mybir.dt.float32r
    bf16 = mybir.dt.bfloat16
    EXP = mybir.ActivationFunctionType.Exp

    nc = bacc.Bacc()
    xT = nc.declare_dram_parameter("xT", [D, T], f32, isOutput=False)
    wqk = nc.declare_dram_parameter("wqk", [D, 4 * P], f32, isOutput=False)
    wv = nc.declare_dram_parameter("wv", [D, 2 * P], f32, isOutput=False)
    wout = nc.declare_dram_parameter("wout", [2 * P, D], bf16, isOutput=False)
    ones = nc.declare_dram_parameter("ones", [P, NTB], bf16, isOutput=False)
    out = nc.declare_dram_parameter("out", [T, D], f32, isOutput=True)

    with tile.TileContext(nc) as tc, ExitStack() as top:
        const = top.enter_context(tc.tile_pool(name="const", bufs=1))
        wout_sb = const.tile([P, 2, D], bf16)  # 4KB/p
        qkT = const.tile([P, 4, T], f32r)  # q0,k0,q1,k1 feature-major, 32KB/p
        V_sb = const.tile([P, NTB, HPC, DK + 1], bf16)  # token-major V, 8.3KB/p
        o_sb = const.tile([P, 2, NTB, P], bf16)  # normalized o token-major, 8KB/p
        oT = const.tile([P, 2, T], bf16)  # feature-major o, 8KB/p
        identb = const.tile([P, P], bf16)
        make_identity(nc, identb[:])

        rep_ctx = tc.For_i(0, reps, 1) if reps > 1 else nullcontext()
        top.enter_context(rep_ctx)

        nc.sync.dma_start(wout_sb[:], wout.rearrange("(c p) n -> p c n", p=P))
        for h in range(HPC):
            nc.sync.dma_start(V_sb[:, :, h, DK : DK + 1], ones[:, :])

        # ---- projection phase (x / weights live only here) ----
        with (
            tc.tile_pool(name="xw", bufs=1) as xw_pool,
            tc.tile_pool(name="psA", bufs=2, space="PSUM") as psA,
            tc.tile_pool(name="psV", bufs=2, space="PSUM") as psV,
        ):
            x_sb = xw_pool.tile([P, NDC, T], f32r)  # 64KB/p
            wqk_sb = xw_pool.tile([P, NDC, 4 * P], f32r)  # 16KB/p
            wv_sb = xw_pool.tile([P, NDC, 2 * P], f32r)  # 8KB/p
            for dc in range(NDC):
                nc.sync.dma_start(
                    wqk_sb[:, dc, :], wqk[dc * P : (dc + 1) * P, :].bitcast(f32r)
                )
                nc.sync.dma_start(
                    x_sb[:, dc, :], xT[dc * P : (dc + 1) * P, :].bitcast(f32r)
                )
                nc.sync.dma_start(
                    wv_sb[:, dc, :], wv[dc * P : (dc + 1) * P, :].bitcast(f32r)
                )

            for tg in range(2):
                for ec in range(4):
                    # qkT[:, ec, tg half] = wqk_chunk^T @ x  (f32r)
                    ps = psA.tile([P, 1024], f32, name="psa", tag="psa")
                    for dc in range(NDC):
                        for hh in range(2):
                            nc.tensor.matmul(
                                ps[:, hh * 512 : (hh + 1) * 512],
                                wqk_sb[:, dc, ec * P : (ec + 1) * P],
                                x_sb[
                                    :,
                                    dc,
                                    tg * 1024 + hh * 512 : tg * 1024 + (hh + 1) * 512,
                                ],
                                start=(dc == 0),
                                stop=(dc == NDC - 1),
                            )
                    nc.vector.tensor_copy(
                        qkT[:, ec, tg * 1024 : (tg + 1) * 1024], ps[:]
                    )
            for tb in range(NTB):
                # V_sb[:, tb, :, 0:64] = x_chunk^T @ wv  (token-major)
                psv = psV.tile([P, 2 * P], f32, name="psv", tag="psv")
                for dc in range(NDC):
                    nc.tensor.matmul(
                        psv[:],
                        x_sb[:, dc, tb * P : (tb + 1) * P],
                        wv_sb[:, dc, :],
                        start=(dc == 0),
                        stop=(dc == NDC - 1),
                    )
                nc.scalar.copy(
                    V_sb[:, tb, :, 0:DK],
                    psv.rearrange("p (h d) -> p h d", h=HPC),
                )

        # ---- attention phase: 8 jobs = (head, query half) ----
        with (
            tc.tile_pool(name="pqk", bufs=2, space="PSUM") as pqk_pool,
            tc.tile_pool(name="pav", bufs=4, space="PSUM") as pav_pool,
            tc.tile_pool(name="pp", bufs=2) as p_pool,
            tc.tile_pool(name="rc", bufs=4) as rec_pool,
        ):
            for h in range(HPC):
                pair, row = h // 2, (h % 2) * DK
                for half in range(2):
                    q0 = half * 1024
                    Pt = p_pool.tile([P, NTB, 1024], bf16, name="pt", tag="pt")
                    for kb in range(NTB):
                        pq = pqk_pool.tile([P, 1024], f32, name="pq", tag="pq")
                        for hh in range(2):
                            nc.tensor.matmul(
                                pq[:, hh * 512 : (hh + 1) * 512],
                                qkT[
                                    row : row + DK, 2 * pair + 1, kb * P : (kb + 1) * P
                                ],
                                qkT[
                                    row : row + DK,
                                    2 * pair,
                                    q0 + hh * 512 : q0 + (hh + 1) * 512,
                                ],
                                start=True,
                                stop=True,
                            )
                        nc.scalar.activation(
                            Pt[:, kb, :], pq[:], EXP, scale=SCALE
                        )
                    for qb in range(8):
                        av = pav_pool.tile([P, 512], f32, name="av", tag="av")
                        for kb in range(NTB):
                            nc.tensor.matmul(
                                av[:, 0 : DK + 1],
                                Pt[:, kb, qb * P : (qb + 1) * P],
                                V_sb[:, kb, h, :],
                                start=(kb == 0),
                                stop=(kb == NTB - 1),
                            )
                        # normalize: per-partition denominator in column DK
                        qg = half * 8 + qb
                        rec = rec_pool.tile([P, 1], f32, name="rec", tag="rec")
                        nc.vector.reciprocal(rec[:], av[:, DK : DK + 1])
                        nc.vector.tensor_scalar_mul(
                            o_sb[:, pair, qg, row : row + DK],
                            av[:, 0:DK],
                            rec[:],
                        )
                if h % 2 == 1:
                    # pair complete: transpose o_sb -> oT (borrow pqk slots)
                    for qb in range(NTB):
                        tr = pqk_pool.tile([P, 1024], f32, name="tr", tag="pq")
                        trb = tr.bitcast(bf16)[:, 0:P]
                        nc.tensor.transpose(trb, o_sb[:, pair, qb, :], identb[:])
                        nc.vector.tensor_copy(
                            oT[:, pair, qb * P : (qb + 1) * P], trb
                        )

        # ---- output projection ----
        with (
            tc.tile_pool(name="pso", bufs=2, space="PSUM") as pso,
            tc.tile_pool(name="osb", bufs=3) as osb_pool,
        ):
            for tb in range(NTB):
                po = pso.tile([P, D], f32, name="po", tag="po")
                for pair in range(2):
                    for nh in range(2):
                        nc.tensor.matmul(
                            po[:, nh * 512 : (nh + 1) * 512],
                            oT[:, pair, tb * P : (tb + 1) * P],
                            wout_sb[:, pair, nh * 512 : (nh + 1) * 512],
                            start=(pair == 0),
                            stop=(pair == 1),
                        )
                ob = osb_pool.tile([P, D], f32, name="ob", tag="ob")
                nc.scalar.copy(ob[:], po[:])
                nc.sync.dma_start(out[tb * P : (tb + 1) * P, :], ob[:])

    nc.compile()
    return nc


def get_program():
    if "nc" not in _CACHE:
        _CACHE["nc"] = _build_program()
    return _CACHE["nc"]


def make_in_maps(query, pos_emb, w_qkv, w_out):
    from ml_dtypes import bfloat16

    query = np.asarray(query, dtype=np.float32)
    pos_emb = np.asarray(pos_emb, dtype=np.float32)
    w_qkv = np.asarray(w_qkv, dtype=np.float32)
    w_out = np.asarray(w_out, dtype=np.float32)
    xTs = [np.ascontiguousarray((query[b] + pos_emb).T) for b in range(B)]
    ones = np.ones((P, NTB), dtype=bfloat16)
    in_maps = []
    for c in range(NCORES):
        b, hb = c // (NCORES // B), (c % (NCORES // B)) * HPC
        heads = list(range(hb, hb + HPC))
        # w_qkv column e for head h, kind j (q/k/v), dim d: e = h*3*DK + j*DK + d
        qk_cols = []
        for pair in range(2):
            for j in range(2):  # q then k
                for h in heads[2 * pair : 2 * pair + 2]:
                    base = h * 3 * DK + j * DK
                    qk_cols.append(w_qkv[:, base : base + DK])
        wqk_c = np.ascontiguousarray(np.concatenate(qk_cols, axis=1))
        wv_c = np.ascontiguousarray(
            np.concatenate(
                [w_qkv[:, h * 3 * DK + 2 * DK : h * 3 * DK + 3 * DK] for h in heads],
                axis=1,
            )
        )
        wout_c = np.concatenate(
            [w_out[h * DK : (h + 1) * DK, :] for h in heads], axis=0
        ).astype(bfloat16)
        in_maps.append(
            {
                "xT": xTs[b],
                "wqk": wqk_c,
                "wv": wv_c,
                "wout": wout_c,
                "ones": ones,
            }
        )
    return in_maps


def gather_output(results):
    out = np.zeros((B, S, D), dtype=np.float32)
    for c in range(NCORES):
        out[c // (NCORES // B)] += results[c]["out"]
    return out


def kernel(query, pos_emb, w_qkv, w_out):
    from concourse.bass_utils import run_bass_kernel_spmd

    nc = get_program()
    in_maps = make_in_maps(query, pos_emb, w_qkv, w_out)
    res = run_bass_kernel_spmd(nc, in_maps, list(range(NCORES)))
    return gather_output(res.results)
